# revision 32
# baseline (speedup 1.0000x reference)
"""Multi-head causal self-attention with RoPE on 8 Trainium2 NeuronCores.

Sharding: DP(2) x TP(4). Cores [4g, 4g+4) own batch g; within a group,
core r owns heads [4r, 4r+4) (rows [r*512,(r+1)*512) of Wq/Wk/Wv and the
matching columns of Wo). The host sums the 4 partial output projections
per batch (replaces the TP all-reduce); partial sums travel as fp16.

Performance notes (measured on TRN2):
  - PE matmul issue rate is N cycles @2.4GHz regardless of operand dtype;
    the kernel is PE-streaming-bound (~280us of columns), so everything
    else is organized to keep the PE FIFO dense.
  - The HAM clock gate runs the PE at 1.2GHz until ~3.4us of sustained
    busy; ~8 dummy matmuls at kernel start warm it up during the initial
    DMA wait (framework preamble ends ~9.2us, first DMA data ~11.4us).
  - dma_start issue costs ~600ns on the Sync queue regardless of size,
    and only 8 HW-DMA semaphores rotate, so transfers are kept >=256KB
    and issued in exact consumption order (wq/x interleaved by c-chunk,
    then wk, wv, xt1, xt2; wv BEFORE xt1 -- the v chains run before the
    si=1 q chains and a late wv costs a 3.9us stall + a HAM re-throttle).
  - s-chunk 0's q/k projections run c-major across 4 concurrent PSUM
    accumulators so the PE starts on the first 256KB of weights/x.
  - Scalar activations pay a ~370-cycle access-latency adder, so exps are
    batched two k-chunks per call; in attention the scalar engine does
    ONLY exps (evictions go to vector/gpsimd) so a score-PSUM tile is
    freed ~780ns after its pair completes and the 3-deep st ring never
    stalls the PE.
  - Softmax denominators: all exp'd chunks of a (head, q-chunk) chain
    are accumulated in fp16 on the DVE, then partition-reduced with a
    single ones-matmul; the reduce/reciprocal/normalize epilogue is
    lagged into the next chain (flushed after its SECOND score pair so
    the DVE accumulation has fully drained), except each block's h=3
    epilogue which flushes at that chain's end so the next block's
    output-projection doses read normalized uT.
  - Attention blocks run qc order [0,3,2,1]: the undosed first block is
    the one with the smallest scalar-exp-over-PE deficit (qc=0,
    diag-only) and needs only s-chunk-0 q/k at the phase boundary.
  - Causality: diagonal-band score chunks are column-trimmed to
    N = 512-128*di and masked multiplicatively (after exp) with a single
    [128,128] triangular 0/1 mask.
"""

import sys

import numpy as np

B, S, DIM = 2, 2048, 2048
NUM_HEADS = 16
HD = 128
N_CORES = 8
DP = 2                       # data-parallel groups (one batch each)
TP = N_CORES // DP           # tensor-parallel ranks per group
HPC = NUM_HEADS // TP        # heads per core (4)
DLOC = HPC * HD              # per-core slice of the model dim (512)
ROPE_BASE = 10000.0
SC = 512                     # s-chunk for projections / attention q-chunk

_PROGRAM_CACHE = {}


def _rope_tables_T(seq_len, head_dim):
    # match reference float32 arithmetic: inv_freq over even indices,
    # emb = cat(freqs, freqs); returned transposed [head_dim, seq_len]
    inv_freq = (
        1.0
        / (np.float32(ROPE_BASE)
           ** (np.arange(0, head_dim, 2, dtype=np.float32) / np.float32(head_dim)))
    ).astype(np.float32)
    t = np.arange(seq_len, dtype=np.float32)
    freqs = np.outer(t, inv_freq).astype(np.float32)      # [S, D/2]
    emb = np.concatenate([freqs, freqs], axis=-1)         # [S, D]
    return (
        np.ascontiguousarray(np.cos(emb).astype(np.float16).T),
        np.ascontiguousarray(np.sin(emb).astype(np.float16).T),
    )


def _tri01():
    # tri01[kk, qq] = 1 if kk <= qq else 0 (multiplicative causal mask for
    # the [128,128] diagonal block of every diagonal k-chunk)
    kk = np.arange(128)[:, None]
    qq = np.arange(128)[None, :]
    return np.ascontiguousarray((kk <= qq).astype(np.float16))


def build_program(s=S, dim=DIM):
    """Per-core SPMD Bass program (identical on every core)."""
    if "/opt/trn_rl_repo" not in sys.path:
        sys.path.insert(0, "/opt/trn_rl_repo")
    import concourse.bacc as bacc
    import concourse.mybir as mybir
    import concourse.tile as tile

    f32 = mybir.dt.float32
    f16 = mybir.dt.float16
    EXP = mybir.ActivationFunctionType.Exp

    n_din = dim // 128          # contraction chunks for projections (16)
    n_sc = s // SC              # s-chunks (4)
    n_oc = dim // 128           # output-projection row chunks (16)
    scale = float(HD) ** -0.5

    nc = bacc.Bacc("TRN2", target_bir_lowering=False, debug=False)

    # all DRAM tensors pre-tiled on the host: partition dim first, then
    # per-partition-contiguous free dims, so DMAs are 128 fat descriptors
    x_d = nc.dram_tensor("x", [128, n_sc, n_din, SC], f16, kind="ExternalInput")
    wq_d = nc.dram_tensor("wq", [128, n_din, DLOC], f16, kind="ExternalInput")
    wk_d = nc.dram_tensor("wk", [128, n_din, DLOC], f16, kind="ExternalInput")
    wv_d = nc.dram_tensor("wv", [128, n_din, DLOC], f16, kind="ExternalInput")
    wo_d = nc.dram_tensor("wo", [128, HPC, dim], f16, kind="ExternalInput")
    cosT_d = nc.dram_tensor("cosT", [HD, s], f16, kind="ExternalInput")
    sinT_d = nc.dram_tensor("sinT", [HD, s], f16, kind="ExternalInput")
    ones_d = nc.dram_tensor("ones", [HD, HD], f16, kind="ExternalInput")
    tri_d = nc.dram_tensor("tri", [HD, HD], f16, kind="ExternalInput")
    out_d = nc.dram_tensor("out", [128, n_sc, n_oc, SC], f16, kind="ExternalOutput")

    with tile.TileContext(nc) as tc:
        with tc.tile_pool(name="persist", bufs=1) as persist:
            qT = persist.tile([128, HPC, s], f16)   # roped q, [d, h, s]
            kT = persist.tile([128, HPC, s], f16)
            vS = persist.tile([128, s // 128, DLOC], f16)  # [k, chunk, d]
            uT = persist.tile([128, HPC, s], f16)   # attention out, [d, h, s]

            # ---------------- phase 1: qkv projections + RoPE ----------------
            with (
                tc.tile_pool(name="p1x", bufs=3) as p1x,
                tc.tile_pool(name="p1w", bufs=1) as p1w,
                tc.tile_pool(name="p1t", bufs=2) as p1t,
                tc.tile_pool(name="ps1", bufs=8, space="PSUM") as ps1,
            ):
                xts = [
                    p1x.tile([128, n_din, SC], f16, tag="xt", name=f"xt{si}")
                    for si in range(3)
                ]
                wq_s = p1w.tile([128, n_din, DLOC], f16)
                wk_s = p1w.tile([128, n_din, DLOC], f16)
                wv_s = p1w.tile([128, n_din, DLOC], f16)
                cosT = persist.tile([HD, s], f16)
                sinT = persist.tile([HD, s], f16)
                ones = persist.tile([HD, HD], f16)
                tri01 = persist.tile([HD, HD], f16)
                woT_s = persist.tile([128, HPC, dim], f16)

                # PE warm-up: the HAM clock gate needs ~3.4us of sustained
                # matmul activity to lift the PE from 1.2 to 2.4GHz.  Burn
                # the dead time between the framework preamble (~9.2us) and
                # the first DMA'd operands (~12.5us) on dummy matmuls so the
                # real stream starts warm.
                warm = p1x.tile([128, SC], f16, tag="warm", bufs=1)
                nc.gpsimd.memset(warm, 0.0)
                for wi in range(8):
                    wacc = ps1.tile([128, SC], f32, tag="acc", name=f"warm{wi}")
                    nc.tensor.matmul(
                        wacc, lhsT=warm[:, :128], rhs=warm, start=True, stop=True
                    )

                # DMA issue order == consumption order.  Each dma_start costs
                # ~600ns of Sync-queue issue, so pieces are >=256KB except the
                # leading ones that gate the very first matmuls.
                def dspan(dst, src, lo, hi):
                    nc.sync.dma_start(out=dst[:, lo:hi, :], in_=src[:, lo:hi, :])

                def dq(a, b):
                    dspan(wq_s, wq_d, a, b)
                    nc.sync.dma_start(
                        out=xts[0][:, a:b, :], in_=x_d[:, 0, a:b, :]
                    )

                dq(0, 2)
                dq(2, 4)
                dq(4, 6)
                dq(6, 8)
                # first s-chunk of the RoPE tables (needed by the first rope
                # at ~25us); the rest comes after wk
                nc.sync.dma_start(out=cosT[:, :SC], in_=cosT_d[:, :SC])
                nc.sync.dma_start(out=sinT[:, :SC], in_=sinT_d[:, :SC])
                dq(8, 10)
                dq(10, 12)
                dq(12, 14)
                dspan(wk_s, wk_d, 0, 2)
                dq(14, 16)
                dspan(wk_s, wk_d, 2, 4)
                dspan(wk_s, wk_d, 4, 6)
                dspan(wk_s, wk_d, 6, 8)
                dspan(wk_s, wk_d, 8, 12)
                dspan(wk_s, wk_d, 12, 16)
                nc.sync.dma_start(out=cosT[:, SC:], in_=cosT_d[:, SC:])
                nc.sync.dma_start(out=sinT[:, SC:], in_=sinT_d[:, SC:])
                dspan(wv_s, wv_d, 0, 8)
                dspan(wv_s, wv_d, 8, 16)
                nh = n_din // 2
                nc.sync.dma_start(out=xts[1][:, :nh, :], in_=x_d[:, 1, :nh, :])
                nc.sync.dma_start(out=xts[1][:, nh:, :], in_=x_d[:, 1, nh:, :])
                nc.sync.dma_start(out=xts[2][:, :nh, :], in_=x_d[:, 2, :nh, :])
                nc.sync.dma_start(out=xts[2][:, nh:, :], in_=x_d[:, 2, nh:, :])
                nc.sync.dma_start(out=ones, in_=ones_d[:])
                nc.sync.dma_start(out=tri01, in_=tri_d[:])
                nc.sync.dma_start(out=woT_s, in_=wo_d[:])

                def finish_rope(raw, store, h, s0):
                    # rotate-half as a partition-permuting SBUF->SBUF DMA
                    # (engines cannot cross partitions; the DMA can), with
                    # the rotation signs folded into the host sin table --
                    # saves a 216ns PE matmul per chain and runs the sin
                    # multiply at 2x fp16 DVE rate. Emitted one chain late
                    # so nothing waits on the scalar-engine raw copy.
                    rawp = p1t.tile([128, SC], f16, tag="rp", bufs=2)
                    nc.sync.dma_start(out=rawp[0:64, :], in_=raw[1:128:2, :])
                    nc.sync.dma_start(out=rawp[64:128, :], in_=raw[0:128:2, :])
                    t1 = p1t.tile([128, SC], f16, tag="t1")
                    nc.vector.tensor_mul(t1, raw, cosT[:, s0 : s0 + SC])
                    t2 = p1t.tile([128, SC], f16, tag="t2")
                    nc.vector.tensor_mul(t2, rawp, sinT[:, s0 : s0 + SC])
                    nc.gpsimd.tensor_add(store[:, h, s0 : s0 + SC], t1, t2)

                pending = None

                # --- si=0: c-major q/k blocks across 4 concurrent PSUM
                # accumulators, so the PE only ever waits on the c-chunk of
                # wq/x currently streaming in (the DMA-paced startup).
                for w_s, store in ((wq_s, qT), (wk_s, kT)):
                    accs = [
                        ps1.tile([128, SC], f32, tag="acc", name=f"p0acc{h}")
                        for h in range(HPC)
                    ]
                    for c in range(n_din):
                        for h in range(HPC):
                            nc.tensor.matmul(
                                accs[h],
                                lhsT=w_s[:, c, h * HD : (h + 1) * HD],
                                rhs=xts[0][:, c, :],
                                start=(c == 0),
                                stop=(c == n_din - 1),
                            )
                    for h in range(HPC):
                        raw = p1t.tile([128, SC], f16, tag="raw", bufs=3)
                        nc.scalar.copy(raw, accs[h])
                        if pending is not None:
                            finish_rope(*pending)
                        pending = (raw, store, h, 0)

                for sub in range(SC // 128):   # si=0 v chains
                    vacc = ps1.tile([128, SC], f32, tag="acc", name="vacc0")
                    for c in range(n_din):
                        nc.tensor.matmul(
                            vacc,
                            lhsT=xts[0][:, c, sub * 128 : (sub + 1) * 128],
                            rhs=wv_s[:, c, :],
                            start=(c == 0),
                            stop=(c == n_din - 1),
                        )
                    if pending is not None:
                        finish_rope(*pending)
                        pending = None
                    vdst = vS[:, sub, :]
                    if sub % 2 == 0:
                        nc.scalar.copy(vdst, vacc)
                    else:
                        nc.vector.tensor_copy(vdst, vacc)

                # --- si=1..3: h-major (weights resident; x tiles streamed)
                for si in range(1, n_sc):
                    s0 = si * SC
                    if si < 3:
                        xt = xts[si]
                    else:
                        xt = p1x.tile([128, n_din, SC], f16, tag="xt", name="xt3")
                        nc.sync.dma_start(out=xt, in_=x_d[:, si, :, :])

                    for w_s, store in ((wq_s, qT), (wk_s, kT)):
                        for h in range(HPC):
                            acc = ps1.tile([128, SC], f32, tag="acc")
                            for c in range(n_din):
                                nc.tensor.matmul(
                                    acc,
                                    lhsT=w_s[:, c, h * HD : (h + 1) * HD],
                                    rhs=xt[:, c, :],
                                    start=(c == 0),
                                    stop=(c == n_din - 1),
                                )
                            raw = p1t.tile([128, SC], f16, tag="raw", bufs=3)
                            nc.scalar.copy(raw, acc)
                            if pending is not None:
                                finish_rope(*pending)
                            pending = (raw, store, h, s0)

                    for sub in range(SC // 128):   # v chains
                        vacc = ps1.tile([128, SC], f32, tag="acc", name="vacc")
                        for c in range(n_din):
                            nc.tensor.matmul(
                                vacc,
                                lhsT=xt[:, c, sub * 128 : (sub + 1) * 128],
                                rhs=wv_s[:, c, :],
                                start=(c == 0),
                                stop=(c == n_din - 1),
                            )
                        if pending is not None:
                            finish_rope(*pending)
                            pending = None
                        vdst = vS[:, si * 4 + sub, :]
                        # si=3: keep the scalar queue clear for the first
                        # attention exps right after the phase boundary
                        if sub % 2 == 0 and si < 3:
                            nc.scalar.copy(vdst, vacc)
                        else:
                            nc.vector.tensor_copy(vdst, vacc)

            # ------------- phase 2+3: attention + output projection -------------
            with (
                tc.tile_pool(name="p2", bufs=8) as p2,
                tc.tile_pool(name="p2l", bufs=2) as p2l,
                tc.tile_pool(name="p2r", bufs=2) as p2r,
                tc.tile_pool(name="p3", bufs=2) as p3,
                tc.tile_pool(name="ps_t", bufs=3, space="PSUM") as ps_t,
                tc.tile_pool(name="ps_o", bufs=2, space="PSUM") as ps_o,
            ):
                def phase3_og_gen(qc, og, n_sc_evict=2):
                    # one 4-row-chunk group of the output projection for
                    # q-chunk qc: 16 PE matmuls + psum evictions + 1 DMA.
                    # Yields after each 4-matmul row-chunk so the group can be
                    # dosed through the next q-chunk's attention chains --
                    # keeping per-pair PE work above the scalar exp rate.
                    # Per-row-chunk psum evictions, alternating engines
                    # (gpsimd cannot read PSUM): half-size ops block the
                    # latency-critical exps/masks in the FIFOs half as long.
                    q0 = qc * SC
                    ot4 = p3.tile([128, 4, SC], f16, tag="ot")
                    for op_ in range(2):
                        pos2 = ps_t.tile([128, 2, SC], f32, tag="st", name="pos")
                        for j in range(2):
                            oc = og * 4 + op_ * 2 + j
                            for h in range(HPC):
                                nc.tensor.matmul(
                                    pos2[:, j, :],
                                    lhsT=woT_s[:, h, oc * 128 : (oc + 1) * 128],
                                    rhs=uT[:, h, q0 : q0 + SC],
                                    start=(h == 0),
                                    stop=(h == HPC - 1),
                                )
                            dst = ot4[:, op_ * 2 + j, :]
                            # row r goes to scalar iff r is among the first
                            # n_sc_evict even rows: the dosing block chooses
                            # the split by its own scalar-exp headroom
                            r = op_ * 2 + j
                            if r % 2 == 0 and r // 2 < n_sc_evict:
                                nc.scalar.copy(dst, pos2[:, j, :])
                            else:
                                nc.vector.tensor_copy(dst, pos2[:, j, :])
                            yield
                    nc.sync.dma_start(
                        out=out_d[:, qc, og * 4 : (og + 1) * 4, :], in_=ot4
                    )

                def phase3_og_final(qc, og, last=False):
                    # final-loop variant (no exps in flight): full-pair
                    # evictions on scalar/vector, DMA split per pair so the
                    # last transfer after the last matmul is only 256KB.
                    # The very last pair is evicted per-oc on BOTH engines in
                    # parallel with per-oc DMAs, halving the kernel tail.
                    q0 = qc * SC
                    ot4 = p3.tile([128, 4, SC], f16, tag="ot")
                    for op_ in range(2):
                        pos2 = ps_t.tile([128, 2, SC], f32, tag="st", name="pos")
                        for j in range(2):
                            oc = og * 4 + op_ * 2 + j
                            for h in range(HPC):
                                nc.tensor.matmul(
                                    pos2[:, j, :],
                                    lhsT=woT_s[:, h, oc * 128 : (oc + 1) * 128],
                                    rhs=uT[:, h, q0 : q0 + SC],
                                    start=(h == 0),
                                    stop=(h == HPC - 1),
                                )
                        dst2 = ot4[:, op_ * 2 : op_ * 2 + 2, :]
                        # vector first, scalar last: the scheduler orders the
                        # second eviction after the first's engine tick, so
                        # put the kernel-tail-critical one on the engine
                        # whose predecessor finished earliest
                        if op_ == 0:
                            nc.vector.tensor_copy(dst2, pos2)
                        else:
                            nc.scalar.copy(dst2, pos2)
                        nc.sync.dma_start(
                            out=out_d[:, qc, og * 4 + op_ * 2 : og * 4 + op_ * 2 + 2, :],
                            in_=dst2,
                        )

                # attention q-chunk order: every chain is paced by
                # max(PE, scalar-exp) work; blocks after the first carry a
                # dosed output-projection group (+3.4us PE) from the
                # previously finished chunk, so the undosed FIRST block
                # should be the one with the smallest scalar-over-PE
                # deficit: qc=0 (diag-only, ~2us exp vs 1.3us PE).  qc=0
                # first also needs only s-chunk-0 q/k, making the
                # phase-1 -> phase-2 handoff dependency-free.  Each chain's
                # epilogue (denominator matmul + reciprocal + normalize) is
                # lagged into the next chain so the PE never waits on the
                # DVE accumulation at head boundaries.
                qc_order = [0] + list(range(n_sc - 1, 0, -1))
                epi_pend = []

                def flush_epi():
                    while epi_pend:
                        outp, lall, h, q0 = epi_pend.pop(0)
                        lrep = ps_t.tile(
                            [128, 2, SC], f32, tag="st", name="lrep"
                        )
                        nc.tensor.matmul(
                            lrep[:, 0, :], lhsT=ones, rhs=lall,
                            start=True, stop=True,
                        )
                        rec = p2r.tile([128, SC], f32, tag="rec")
                        nc.vector.reciprocal_approx_fast(rec, lrep[:, 0, :])
                        nc.vector.tensor_mul(uT[:, h, q0 : q0 + SC], outp, rec)

                for oi, qc in enumerate(qc_order):
                    q0 = qc * SC
                    nfull = 4 * qc          # full (sub-diagonal) k-chunks
                    nkc = nfull + 4
                    prev_qc = qc_order[oi - 1] if oi > 0 else None
                    for h in range(HPC):
                        outp = ps_o.tile([128, SC], f32, tag="o")
                        lall = p2l.tile([128, SC], f16, tag="lp")
                        lst = [False]       # lall initialized?
                        pend_av = []        # (kc, pt AP, co) awaiting AV
                        og_gen = (
                            # the qc=3 block's chains have the least scalar
                            # headroom (8.6us of exps vs 9.8us PE): evict
                            # only 1 of 4 rows on scalar there
                            phase3_og_gen(prev_qc, h, 1 if qc == 3 else 2)
                            if prev_qc is not None
                            else None
                        )

                        def dose():
                            # a slice of the previous q-chunk's output
                            # projection as PE filler between score pairs
                            if og_gen is not None:
                                next(og_gen, None)

                        def mid_chain():
                            # previous chain's epilogue; deferred to the
                            # SECOND score pair so the previous chain's DVE
                            # denominator accumulation has fully drained.
                            # (Safe for the og dose at pr==0: each block's
                            # h=3 epilogue is flushed at that chain's end,
                            # so all uT heads of the previous q-chunk are
                            # normalized before the next block's doses.)
                            flush_epi()
                            dose()

                        def lacc(ap, co):
                            # fp16 DVE accumulation of the softmax denominator
                            if not lst[0]:
                                nc.vector.tensor_copy(lall, ap)
                                lst[0] = True
                            else:
                                nc.vector.tensor_add(
                                    lall[:, co:], lall[:, co:], ap
                                )

                        def flush_av(upto):
                            # AV matmuls lag the score/exp stream to keep exp
                            # latency off the PE critical path
                            while len(pend_av) > upto:
                                kc, pt_ap, co = pend_av.pop(0)
                                nc.tensor.matmul(
                                    outp[:, co:],
                                    lhsT=vS[:, kc, h * HD : (h + 1) * HD],
                                    rhs=pt_ap,
                                    start=(kc == 0),
                                    stop=(kc == nkc - 1),
                                )

                        # --- full chunks, exp'd in pairs ---
                        for pr in range(nfull // 2):
                            st2 = ps_t.tile([128, 2, SC], f32, tag="st")
                            pt2 = p2.tile([128, 2, SC], f16, tag="pt")
                            for j in range(2):
                                kc = pr * 2 + j
                                nc.tensor.matmul(
                                    st2[:, j, :],
                                    lhsT=kT[:, h, kc * 128 : (kc + 1) * 128],
                                    rhs=qT[:, h, q0 : q0 + SC],
                                    start=True,
                                    stop=True,
                                )
                            nc.scalar.activation(pt2, st2, EXP, scale=scale)
                            # NOTE: offloading the pair-sum to gpsimd was
                            # tried and reverted -- gpsimd needs ~1.15us per
                            # [128,512] add vs the 860ns pair cadence, so it
                            # falls behind and the in-order DVE queue ends up
                            # head-blocked on it (+6us).
                            if pr == 1:
                                mid_chain()
                            else:
                                dose()
                            if lst[0]:
                                nc.vector.tensor_add(lall, lall, pt2[:, 0, :])
                            else:
                                nc.vector.tensor_add(
                                    lall, pt2[:, 0, :], pt2[:, 1, :]
                                )
                                lst[0] = True
                            if pr > 0:
                                nc.vector.tensor_add(lall, lall, pt2[:, 1, :])
                            pend_av.append((pr * 2, pt2[:, 0, :], 0))
                            pend_av.append((pr * 2 + 1, pt2[:, 1, :], 0))
                            flush_av(4)

                        # --- diagonal chunks, trimmed + masked ---
                        for di in range(4):
                            kc = nfull + di
                            co = 128 * di
                            st2 = ps_t.tile([128, 2, SC], f32, tag="st")
                            pt2 = p2.tile([128, 2, SC], f16, tag="pt")
                            nc.tensor.matmul(
                                st2[:, 0, co:],
                                lhsT=kT[:, h, kc * 128 : (kc + 1) * 128],
                                rhs=qT[:, h, q0 + co : q0 + SC],
                                start=True,
                                stop=True,
                            )
                            nc.scalar.activation(
                                pt2[:, 0, co:], st2[:, 0, co:], EXP, scale=scale
                            )
                            # causal mask issued BEFORE the dose: otherwise
                            # it queues behind the dosed og eviction on the
                            # DVE and stalls the AV matmul ~760ns per chain
                            # (gpsimd is worse: ~400-600ns op latency)
                            nc.vector.tensor_mul(
                                pt2[:, 0, co : co + 128],
                                pt2[:, 0, co : co + 128],
                                tri01,
                            )
                            if nfull == 0 and di == 1:
                                mid_chain()
                            else:
                                dose()
                            lacc(pt2[:, 0, co:], co)
                            pend_av.append((kc, pt2[:, 0, co:], co))
                            flush_av(4)
                        flush_av(0)
                        if og_gen is not None:
                            for _ in og_gen:
                                pass
                        epi_pend.append((outp, lall, h, q0))
                        if h == HPC - 1:
                            # flush the block's last epilogue here (the
                            # trailing AV flush + og drain above give the
                            # DVE time to finish lall), so the next block's
                            # output-projection doses read normalized uT
                            flush_epi()
                flush_epi()
                n_og = n_oc // 4
                for og in range(n_og):
                    phase3_og_final(qc_order[-1], og, last=(og == n_og - 1))

    nc.compile()
    return nc


def make_in_maps(x, Wq, Wk, Wv, Wo):
    cosT, sinT = _rope_tables_T(S, HD)
    # rotate-half signs folded into the sin table: t2[d] = raw[perm(d)]*sin'[d]
    sinT_mod = sinT.copy()
    sinT_mod[:64] *= np.float16(-1)
    ones = np.ones((HD, HD), dtype=np.float16)
    tri = _tri01()
    n_din, n_sc = DIM // 128, S // SC
    xts = []
    for g in range(DP):
        xT = x[g].T.astype(np.float16)                      # [din, s]
        xts.append(np.ascontiguousarray(
            xT.reshape(n_din, 128, n_sc, SC).transpose(1, 2, 0, 3)
        ))                                                  # [128, si, c, j]
    in_maps = []
    for c in range(N_CORES):
        g, r = divmod(c, TP)
        sl = slice(r * DLOC, (r + 1) * DLOC)

        def tile_w_c(W):
            # [p, c, d] = W.T[c*128+p, d]  (c-major; d = h*HD+dd local dim)
            wT = W[sl, :].T.astype(np.float16)              # [din, dloc]
            return np.ascontiguousarray(
                wT.reshape(n_din, 128, DLOC).transpose(1, 0, 2)
            )

        woT = Wo[:, sl].T.astype(np.float16)                # [dloc, dim]
        wo_t = np.ascontiguousarray(
            woT.reshape(HPC, 128, DIM).transpose(1, 0, 2)
        )
        in_maps.append(
            {
                "x": xts[g],
                "wq": tile_w_c(Wq),
                "wk": tile_w_c(Wk),
                "wv": tile_w_c(Wv),
                "wo": wo_t,
                "cosT": cosT,
                "sinT": sinT_mod,
                "ones": ones,
                "tri": tri,
            }
        )
    return in_maps


def kernel(x, Wq, Wk, Wv, Wo, _trace=False):
    """Full-input / full-output entry point. Shards over 8 cores internally."""
    if "/opt/trn_rl_repo" not in sys.path:
        sys.path.insert(0, "/opt/trn_rl_repo")
    from concourse.bass_utils import run_bass_kernel_spmd

    x = np.asarray(x, dtype=np.float32)
    Wq, Wk, Wv, Wo = (np.asarray(w, dtype=np.float32) for w in (Wq, Wk, Wv, Wo))

    key = (B, S, DIM)
    if key not in _PROGRAM_CACHE:
        _PROGRAM_CACHE[key] = build_program(S, DIM)
    nc = _PROGRAM_CACHE[key]

    in_maps = make_in_maps(x, Wq, Wk, Wv, Wo)
    res = run_bass_kernel_spmd(
        nc, in_maps, core_ids=list(range(N_CORES)), trace=_trace
    )
    kernel.last_results = res
    out = np.empty((B, S, DIM), dtype=np.float32)
    for g in range(DP):
        acc = res.results[g * TP]["out"].astype(np.float32)
        for r in range(1, TP):
            acc = acc + res.results[g * TP + r]["out"].astype(np.float32)
        # [128, qc, oc, j] -> [oc*128, qc*512]
        outT = acc.transpose(2, 0, 1, 3).reshape(DIM, S)
        out[g] = outT.T
    return out


# revision 36
# speedup vs baseline: 1.1861x; 1.1861x over previous
"""Multi-head causal self-attention with RoPE on 8 Trainium2 NeuronCores.

Sharding: DP(2) x TP(4). Cores [4g, 4g+4) own batch g; within a group,
core r owns heads [4r, 4r+4) (rows [r*512,(r+1)*512) of Wq/Wk/Wv and the
matching columns of Wo). The host sums the 4 partial output projections
per batch (replaces the TP all-reduce); partial sums travel as fp16.

Performance notes (measured on TRN2):
  - PE matmul issue rate is N cycles @2.4GHz regardless of operand dtype;
    the kernel is PE-streaming-bound (~280us of columns), so everything
    else is organized to keep the PE FIFO dense.
  - The HAM clock gate runs the PE at 1.2GHz until ~3.4us of sustained
    busy; ~8 dummy matmuls at kernel start warm it up during the initial
    DMA wait (framework preamble ends ~9.2us, first DMA data ~11.4us).
  - dma_start issue costs ~600ns on the Sync queue regardless of size,
    and only 8 HW-DMA semaphores rotate, so transfers are kept >=256KB
    and issued in exact consumption order (wq/x interleaved by c-chunk,
    then wk, wv, xt1, xt2; wv BEFORE xt1 -- the v chains run before the
    si=1 q chains and a late wv costs a 3.9us stall + a HAM re-throttle).
  - s-chunk 0's q/k projections run c-major across 4 concurrent PSUM
    accumulators so the PE starts on the first 256KB of weights/x.
  - Scalar activations pay a ~370-cycle access-latency adder, so exps are
    batched two k-chunks per call; in attention the scalar engine does
    ONLY exps (evictions go to vector/gpsimd) so a score-PSUM tile is
    freed ~780ns after its pair completes and the 3-deep st ring never
    stalls the PE.
  - Softmax denominators: all exp'd chunks of a (head, q-chunk) chain
    are accumulated in fp16 on the DVE, then partition-reduced with a
    single ones-matmul; the reduce/reciprocal/normalize epilogue is
    lagged into the next chain (flushed after its SECOND score pair so
    the DVE accumulation has fully drained), except each block's h=3
    epilogue which flushes at that chain's end so the next block's
    output-projection doses read normalized uT.
  - Attention blocks run qc order [0,3,2,1]: the undosed first block is
    the one with the smallest scalar-exp-over-PE deficit (qc=0,
    diag-only) and needs only s-chunk-0 q/k at the phase boundary.
  - Causality: diagonal-band score chunks are column-trimmed to
    N = 512-128*di and masked multiplicatively (after exp) with a single
    [128,128] triangular 0/1 mask.
"""

import sys

import numpy as np

B, S, DIM = 2, 2048, 2048
NUM_HEADS = 16
HD = 128
N_CORES = 8
DP = 2                       # data-parallel groups (one batch each)
TP = N_CORES // DP           # tensor-parallel ranks per group
HPC = NUM_HEADS // TP        # heads per core (4)
DLOC = HPC * HD              # per-core slice of the model dim (512)
ROPE_BASE = 10000.0
SC = 512                     # s-chunk for projections / attention q-chunk

_PROGRAM_CACHE = {}


def _rope_tables_T(seq_len, head_dim):
    # match reference float32 arithmetic: inv_freq over even indices,
    # emb = cat(freqs, freqs); returned transposed [head_dim, seq_len]
    inv_freq = (
        1.0
        / (np.float32(ROPE_BASE)
           ** (np.arange(0, head_dim, 2, dtype=np.float32) / np.float32(head_dim)))
    ).astype(np.float32)
    t = np.arange(seq_len, dtype=np.float32)
    freqs = np.outer(t, inv_freq).astype(np.float32)      # [S, D/2]
    emb = np.concatenate([freqs, freqs], axis=-1)         # [S, D]
    return (
        np.ascontiguousarray(np.cos(emb).astype(np.float16).T),
        np.ascontiguousarray(np.sin(emb).astype(np.float16).T),
    )


def _tri01():
    # tri01[kk, qq] = 1 if kk <= qq else 0 (multiplicative causal mask for
    # the [128,128] diagonal block of every diagonal k-chunk)
    kk = np.arange(128)[:, None]
    qq = np.arange(128)[None, :]
    return np.ascontiguousarray((kk <= qq).astype(np.float16))


def build_program(s=S, dim=DIM):
    """Per-core SPMD Bass program (identical on every core)."""
    if "/opt/trn_rl_repo" not in sys.path:
        sys.path.insert(0, "/opt/trn_rl_repo")
    import concourse.bacc as bacc
    import concourse.mybir as mybir
    import concourse.tile as tile

    f32 = mybir.dt.float32
    f16 = mybir.dt.float16
    EXP = mybir.ActivationFunctionType.Exp

    n_din = dim // 128          # contraction chunks for projections (16)
    n_sc = s // SC              # s-chunks (4)
    n_oc = dim // 128           # output-projection row chunks (16)
    scale = float(HD) ** -0.5

    nc = bacc.Bacc("TRN2", target_bir_lowering=False, debug=False)

    # all DRAM tensors pre-tiled on the host: partition dim first, then
    # per-partition-contiguous free dims, so DMAs are 128 fat descriptors
    x_d = nc.dram_tensor("x", [128, n_sc, n_din, SC], f16, kind="ExternalInput")
    wq_d = nc.dram_tensor("wq", [128, n_din, DLOC], f16, kind="ExternalInput")
    wk_d = nc.dram_tensor("wk", [128, n_din, DLOC], f16, kind="ExternalInput")
    wv_d = nc.dram_tensor("wv", [128, n_din, DLOC], f16, kind="ExternalInput")
    wo_d = nc.dram_tensor("wo", [128, HPC, dim], f16, kind="ExternalInput")
    cosT_d = nc.dram_tensor("cosT", [HD, s], f16, kind="ExternalInput")
    sinT_d = nc.dram_tensor("sinT", [HD, s], f16, kind="ExternalInput")
    ones_d = nc.dram_tensor("ones", [HD, HD], f16, kind="ExternalInput")
    tri_d = nc.dram_tensor("tri", [HD, HD], f16, kind="ExternalInput")
    out_d = nc.dram_tensor("out", [128, n_sc, n_oc, SC], f16, kind="ExternalOutput")

    with tile.TileContext(nc) as tc:
        with tc.tile_pool(name="persist", bufs=1) as persist:
            qT = persist.tile([128, HPC, s], f16)   # roped q, [d, h, s]
            kT = persist.tile([128, HPC, s], f16)
            vS = persist.tile([128, s // 128, DLOC], f16)  # [k, chunk, d]
            uT = persist.tile([128, HPC, s], f16)   # attention out, [d, h, s]

            # ---------------- phase 1: qkv projections + RoPE ----------------
            with (
                tc.tile_pool(name="p1x", bufs=3) as p1x,
                tc.tile_pool(name="p1w", bufs=1) as p1w,
                tc.tile_pool(name="p1t", bufs=2) as p1t,
                tc.tile_pool(name="ps1", bufs=8, space="PSUM") as ps1,
            ):
                xts = [
                    p1x.tile([128, n_din, SC], f16, tag="xt", name=f"xt{si}")
                    for si in range(3)
                ]
                wq_s = p1w.tile([128, n_din, DLOC], f16)
                wk_s = p1w.tile([128, n_din, DLOC], f16)
                wv_s = p1w.tile([128, n_din, DLOC], f16)
                cosT = persist.tile([HD, s], f16)
                sinT = persist.tile([HD, s], f16)
                ones = persist.tile([HD, HD], f16)
                tri01 = persist.tile([HD, HD], f16)
                woT_s = persist.tile([128, HPC, dim], f16)

                # PE warm-up: the HAM clock gate needs ~3.4us of sustained
                # matmul activity to lift the PE from 1.2 to 2.4GHz.  Burn
                # the dead time between the framework preamble (~9.2us) and
                # the first DMA'd operands (~12.5us) on dummy matmuls so the
                # real stream starts warm.
                warm = p1x.tile([128, SC], f16, tag="warm", bufs=1)
                nc.gpsimd.memset(warm, 0.0)
                for wi in range(8):
                    wacc = ps1.tile([128, SC], f32, tag="acc", name=f"warm{wi}")
                    nc.tensor.matmul(
                        wacc, lhsT=warm[:, :128], rhs=warm, start=True, stop=True
                    )

                # DMA issue order == consumption order.  Each dma_start costs
                # ~600ns of Sync-queue issue, so pieces are >=256KB except the
                # leading ones that gate the very first matmuls.
                def dspan(dst, src, lo, hi):
                    nc.sync.dma_start(out=dst[:, lo:hi, :], in_=src[:, lo:hi, :])

                def dq(a, b):
                    dspan(wq_s, wq_d, a, b)
                    nc.sync.dma_start(
                        out=xts[0][:, a:b, :], in_=x_d[:, 0, a:b, :]
                    )

                dq(0, 1)
                dq(1, 2)
                dq(2, 3)
                dq(3, 4)
                dq(4, 6)
                dq(6, 8)
                # first s-chunk of the RoPE tables (needed by the first rope
                # at ~25us); the rest comes after wk
                nc.sync.dma_start(out=cosT[:, :SC], in_=cosT_d[:, :SC])
                nc.sync.dma_start(out=sinT[:, :SC], in_=sinT_d[:, :SC])
                dq(8, 10)
                dq(10, 12)
                dq(12, 14)
                dspan(wk_s, wk_d, 0, 2)
                dq(14, 16)
                dspan(wk_s, wk_d, 2, 4)
                dspan(wk_s, wk_d, 4, 6)
                dspan(wk_s, wk_d, 6, 8)
                dspan(wk_s, wk_d, 8, 12)
                dspan(wk_s, wk_d, 12, 16)
                nc.sync.dma_start(out=cosT[:, SC:], in_=cosT_d[:, SC:])
                nc.sync.dma_start(out=sinT[:, SC:], in_=sinT_d[:, SC:])
                dspan(wv_s, wv_d, 0, 8)
                dspan(wv_s, wv_d, 8, 16)
                nh = n_din // 2
                nc.sync.dma_start(out=xts[1][:, :nh, :], in_=x_d[:, 1, :nh, :])
                nc.sync.dma_start(out=xts[1][:, nh:, :], in_=x_d[:, 1, nh:, :])
                nc.sync.dma_start(out=xts[2][:, :nh, :], in_=x_d[:, 2, :nh, :])
                nc.sync.dma_start(out=xts[2][:, nh:, :], in_=x_d[:, 2, nh:, :])
                nc.sync.dma_start(out=ones, in_=ones_d[:])
                nc.sync.dma_start(out=tri01, in_=tri_d[:])
                nc.sync.dma_start(out=woT_s, in_=wo_d[:])

                def finish_rope(raw, store, h, s0):
                    # rotate-half as a partition-permuting SBUF->SBUF DMA
                    # (engines cannot cross partitions; the DMA can), with
                    # the rotation signs folded into the host sin table --
                    # saves a 216ns PE matmul per chain and runs the sin
                    # multiply at 2x fp16 DVE rate. Emitted one chain late
                    # so nothing waits on the scalar-engine raw copy.
                    rawp = p1t.tile([128, SC], f16, tag="rp", bufs=2)
                    nc.sync.dma_start(out=rawp[0:64, :], in_=raw[1:128:2, :])
                    nc.sync.dma_start(out=rawp[64:128, :], in_=raw[0:128:2, :])
                    t1 = p1t.tile([128, SC], f16, tag="t1")
                    nc.vector.tensor_mul(t1, raw, cosT[:, s0 : s0 + SC])
                    t2 = p1t.tile([128, SC], f16, tag="t2")
                    nc.vector.tensor_mul(t2, rawp, sinT[:, s0 : s0 + SC])
                    nc.gpsimd.tensor_add(store[:, h, s0 : s0 + SC], t1, t2)

                pending = None

                # --- si=0: c-major q/k blocks across 4 concurrent PSUM
                # accumulators, so the PE only ever waits on the c-chunk of
                # wq/x currently streaming in (the DMA-paced startup).
                for w_s, store in ((wq_s, qT), (wk_s, kT)):
                    accs = [
                        ps1.tile([128, SC], f32, tag="acc", name=f"p0acc{h}")
                        for h in range(HPC)
                    ]
                    for c in range(n_din):
                        for h in range(HPC):
                            nc.tensor.matmul(
                                accs[h],
                                lhsT=w_s[:, c, h * HD : (h + 1) * HD],
                                rhs=xts[0][:, c, :],
                                start=(c == 0),
                                stop=(c == n_din - 1),
                            )
                    for h in range(HPC):
                        raw = p1t.tile([128, SC], f16, tag="raw", bufs=3)
                        nc.scalar.copy(raw, accs[h])
                        if pending is not None:
                            finish_rope(*pending)
                        pending = (raw, store, h, 0)

                for sub in range(SC // 128):   # si=0 v chains
                    vacc = ps1.tile([128, SC], f32, tag="acc", name="vacc0")
                    for c in range(n_din):
                        nc.tensor.matmul(
                            vacc,
                            lhsT=xts[0][:, c, sub * 128 : (sub + 1) * 128],
                            rhs=wv_s[:, c, :],
                            start=(c == 0),
                            stop=(c == n_din - 1),
                        )
                    if pending is not None:
                        finish_rope(*pending)
                        pending = None
                    vdst = vS[:, sub, :]
                    if sub % 2 == 0:
                        nc.scalar.copy(vdst, vacc)
                    else:
                        nc.vector.tensor_copy(vdst, vacc)

                # --- si=1..3: h-major (weights resident; x tiles streamed)
                for si in range(1, n_sc):
                    s0 = si * SC
                    if si < 3:
                        xt = xts[si]
                    else:
                        xt = p1x.tile([128, n_din, SC], f16, tag="xt", name="xt3")
                        nc.sync.dma_start(out=xt, in_=x_d[:, si, :, :])

                    # si=3 runs k before q: the PSUM slots that phase 2's
                    # score tiles alias are then last owned by the q accs,
                    # whose scalar raw-copies (the slot releasers) complete
                    # ~10us before the phase boundary instead of right at it
                    qk = ((wq_s, qT), (wk_s, kT))
                    if si == 3:
                        qk = ((wk_s, kT), (wq_s, qT))
                    for w_s, store in qk:
                        for h in range(HPC):
                            acc = ps1.tile([128, SC], f32, tag="acc")
                            for c in range(n_din):
                                nc.tensor.matmul(
                                    acc,
                                    lhsT=w_s[:, c, h * HD : (h + 1) * HD],
                                    rhs=xt[:, c, :],
                                    start=(c == 0),
                                    stop=(c == n_din - 1),
                                )
                            raw = p1t.tile([128, SC], f16, tag="raw", bufs=3)
                            nc.scalar.copy(raw, acc)
                            if pending is not None:
                                finish_rope(*pending)
                            pending = (raw, store, h, s0)

                    for sub in range(SC // 128):   # v chains
                        vacc = ps1.tile([128, SC], f32, tag="acc", name="vacc")
                        for c in range(n_din):
                            nc.tensor.matmul(
                                vacc,
                                lhsT=xt[:, c, sub * 128 : (sub + 1) * 128],
                                rhs=wv_s[:, c, :],
                                start=(c == 0),
                                stop=(c == n_din - 1),
                            )
                        if pending is not None:
                            finish_rope(*pending)
                            pending = None
                        vdst = vS[:, si * 4 + sub, :]
                        # si=3: keep the scalar queue clear for the first
                        # attention exps right after the phase boundary
                        if sub % 2 == 0 and si < 3:
                            nc.scalar.copy(vdst, vacc)
                        else:
                            nc.vector.tensor_copy(vdst, vacc)

            # ------------- phase 2+3: attention + output projection -------------
            with (
                tc.tile_pool(name="p2", bufs=8) as p2,
                tc.tile_pool(name="p2l", bufs=2) as p2l,
                tc.tile_pool(name="p2r", bufs=2) as p2r,
                tc.tile_pool(name="p3", bufs=2) as p3,
                tc.tile_pool(name="ps_t", bufs=3, space="PSUM") as ps_t,
                tc.tile_pool(name="ps_o", bufs=2, space="PSUM") as ps_o,
            ):
                def phase3_og_gen(qc, og, n_sc_evict=2):
                    # one 4-row-chunk group of the output projection for
                    # q-chunk qc: 16 PE matmuls + psum evictions + 1 DMA.
                    # Yields after each 4-matmul row-chunk so the group can be
                    # dosed through the next q-chunk's attention chains --
                    # keeping per-pair PE work above the scalar exp rate.
                    # Per-row-chunk psum evictions, alternating engines
                    # (gpsimd cannot read PSUM): half-size ops block the
                    # latency-critical exps/masks in the FIFOs half as long.
                    q0 = qc * SC
                    ot4 = p3.tile([128, 4, SC], f16, tag="ot")
                    for op_ in range(2):
                        pos2 = ps_t.tile([128, 2, SC], f32, tag="st", name="pos")
                        for j in range(2):
                            oc = og * 4 + op_ * 2 + j
                            for h in range(HPC):
                                nc.tensor.matmul(
                                    pos2[:, j, :],
                                    lhsT=woT_s[:, h, oc * 128 : (oc + 1) * 128],
                                    rhs=uT[:, h, q0 : q0 + SC],
                                    start=(h == 0),
                                    stop=(h == HPC - 1),
                                )
                            dst = ot4[:, op_ * 2 + j, :]
                            # row r goes to scalar iff r is among the first
                            # n_sc_evict even rows: the dosing block chooses
                            # the split by its own scalar-exp headroom
                            r = op_ * 2 + j
                            if r % 2 == 0 and r // 2 < n_sc_evict:
                                nc.scalar.copy(dst, pos2[:, j, :])
                            else:
                                nc.vector.tensor_copy(dst, pos2[:, j, :])
                            yield
                    nc.sync.dma_start(
                        out=out_d[:, qc, og * 4 : (og + 1) * 4, :], in_=ot4
                    )

                def phase3_og_final(qc, og, last=False):
                    # final-loop variant (no exps in flight): full-pair
                    # evictions on scalar/vector, DMA split per pair so the
                    # last transfer after the last matmul is only 256KB.
                    # The very last pair is evicted per-oc on BOTH engines in
                    # parallel with per-oc DMAs, halving the kernel tail.
                    q0 = qc * SC
                    ot4 = p3.tile([128, 4, SC], f16, tag="ot")
                    for op_ in range(2):
                        pos2 = ps_t.tile([128, 2, SC], f32, tag="st", name="pos")
                        for j in range(2):
                            oc = og * 4 + op_ * 2 + j
                            for h in range(HPC):
                                nc.tensor.matmul(
                                    pos2[:, j, :],
                                    lhsT=woT_s[:, h, oc * 128 : (oc + 1) * 128],
                                    rhs=uT[:, h, q0 : q0 + SC],
                                    start=(h == 0),
                                    stop=(h == HPC - 1),
                                )
                        dst2 = ot4[:, op_ * 2 : op_ * 2 + 2, :]
                        # vector first, scalar last: the scheduler orders the
                        # second eviction after the first's engine tick, so
                        # put the kernel-tail-critical one on the engine
                        # whose predecessor finished earliest
                        if op_ == 0:
                            nc.vector.tensor_copy(dst2, pos2)
                        else:
                            nc.scalar.copy(dst2, pos2)
                        nc.sync.dma_start(
                            out=out_d[:, qc, og * 4 + op_ * 2 : og * 4 + op_ * 2 + 2, :],
                            in_=dst2,
                        )

                # attention q-chunk order: every chain is paced by
                # max(PE, scalar-exp) work; blocks after the first carry a
                # dosed output-projection group (+3.4us PE) from the
                # previously finished chunk, so the undosed FIRST block
                # should be the one with the smallest scalar-over-PE
                # deficit: qc=0 (diag-only, ~2us exp vs 1.3us PE).  qc=0
                # first also needs only s-chunk-0 q/k, making the
                # phase-1 -> phase-2 handoff dependency-free.  Each chain's
                # epilogue (denominator matmul + reciprocal + normalize) is
                # lagged into the next chain so the PE never waits on the
                # DVE accumulation at head boundaries.
                qc_order = [0] + list(range(n_sc - 1, 0, -1))
                epi_pend = []

                def flush_epi():
                    while epi_pend:
                        outp, lall, h, q0 = epi_pend.pop(0)
                        lrep = ps_t.tile(
                            [128, 2, SC], f32, tag="st", name="lrep"
                        )
                        nc.tensor.matmul(
                            lrep[:, 0, :], lhsT=ones, rhs=lall,
                            start=True, stop=True,
                        )
                        rec = p2r.tile([128, SC], f32, tag="rec")
                        nc.vector.reciprocal_approx_fast(rec, lrep[:, 0, :])
                        nc.vector.tensor_mul(uT[:, h, q0 : q0 + SC], outp, rec)

                def qc0_paired_block():
                    # qc=0 (diag-only, undosed, first): chains have ~1.3us
                    # of PE work vs ~2us of exps, so run them PAIRWISE --
                    # two chains' score matmuls and exps pipeline against
                    # each other instead of serializing the exp latency.
                    q0, nkc = 0, 4
                    for hp in (0, 2):
                        state = []
                        for h in (hp, hp + 1):
                            state.append((
                                h,
                                ps_o.tile(
                                    [128, SC], f32, tag="o", name=f"o0_{h}"
                                ),
                                p2l.tile(
                                    [128, SC], f16, tag="lp", name=f"lp0_{h}"
                                ),
                            ))
                        pend = {0: [], 1: []}
                        for di in range(4):
                            co = 128 * di
                            pts = {}
                            for ci, (h, _, _) in enumerate(state):
                                st2 = ps_t.tile([128, 2, SC], f32, tag="st")
                                pt2 = p2.tile([128, 2, SC], f16, tag="pt")
                                nc.tensor.matmul(
                                    st2[:, 0, co:],
                                    lhsT=kT[:, h, di * 128 : (di + 1) * 128],
                                    rhs=qT[:, h, q0 + co : q0 + SC],
                                    start=True,
                                    stop=True,
                                )
                                nc.scalar.activation(
                                    pt2[:, 0, co:], st2[:, 0, co:],
                                    EXP, scale=scale,
                                )
                                nc.vector.tensor_mul(
                                    pt2[:, 0, co : co + 128],
                                    pt2[:, 0, co : co + 128],
                                    tri01,
                                )
                                pts[ci] = pt2
                            if di == 1:
                                flush_epi()
                            for ci, (h, _, lall) in enumerate(state):
                                if di == 0:
                                    nc.vector.tensor_copy(
                                        lall, pts[ci][:, 0, :]
                                    )
                                else:
                                    nc.vector.tensor_add(
                                        lall[:, co:], lall[:, co:],
                                        pts[ci][:, 0, co:],
                                    )
                                pend[ci].append((di, pts[ci][:, 0, co:], co))
                        for ci, (h, outp, lall) in enumerate(state):
                            for kc, ap, co in pend[ci]:
                                nc.tensor.matmul(
                                    outp[:, co:],
                                    lhsT=vS[:, kc, h * HD : (h + 1) * HD],
                                    rhs=ap,
                                    start=(kc == 0),
                                    stop=(kc == nkc - 1),
                                )
                            epi_pend.append((outp, lall, h, q0))
                        if hp == 2:
                            # block-end flush so the next block's og doses
                            # read normalized uT
                            flush_epi()

                for oi, qc in enumerate(qc_order):
                    q0 = qc * SC
                    nfull = 4 * qc          # full (sub-diagonal) k-chunks
                    nkc = nfull + 4
                    prev_qc = qc_order[oi - 1] if oi > 0 else None
                    if oi == 0:
                        qc0_paired_block()
                        continue
                    for h in range(HPC):
                        outp = ps_o.tile([128, SC], f32, tag="o")
                        lall = p2l.tile([128, SC], f16, tag="lp")
                        lst = [False]       # lall initialized?
                        pend_av = []        # (kc, pt AP, co) awaiting AV
                        og_gen = (
                            # the qc=3 block's chains have the least scalar
                            # headroom (8.6us of exps vs 9.8us PE): evict
                            # only 1 of 4 rows on scalar there
                            phase3_og_gen(prev_qc, h, 1 if qc == 3 else 2)
                            if prev_qc is not None
                            else None
                        )

                        def dose():
                            # a slice of the previous q-chunk's output
                            # projection as PE filler between score pairs
                            if og_gen is not None:
                                next(og_gen, None)

                        def mid_chain():
                            # previous chain's epilogue; deferred to the
                            # SECOND score pair so the previous chain's DVE
                            # denominator accumulation has fully drained.
                            # (Safe for the og dose at pr==0: each block's
                            # h=3 epilogue is flushed at that chain's end,
                            # so all uT heads of the previous q-chunk are
                            # normalized before the next block's doses.)
                            flush_epi()
                            dose()

                        def lacc(ap, co):
                            # fp16 DVE accumulation of the softmax denominator
                            if not lst[0]:
                                nc.vector.tensor_copy(lall, ap)
                                lst[0] = True
                            else:
                                nc.vector.tensor_add(
                                    lall[:, co:], lall[:, co:], ap
                                )

                        def flush_av(upto):
                            # AV matmuls lag the score/exp stream to keep exp
                            # latency off the PE critical path
                            while len(pend_av) > upto:
                                kc, pt_ap, co = pend_av.pop(0)
                                nc.tensor.matmul(
                                    outp[:, co:],
                                    lhsT=vS[:, kc, h * HD : (h + 1) * HD],
                                    rhs=pt_ap,
                                    start=(kc == 0),
                                    stop=(kc == nkc - 1),
                                )

                        # --- full chunks, exp'd in pairs ---
                        for pr in range(nfull // 2):
                            st2 = ps_t.tile([128, 2, SC], f32, tag="st")
                            pt2 = p2.tile([128, 2, SC], f16, tag="pt")
                            for j in range(2):
                                kc = pr * 2 + j
                                nc.tensor.matmul(
                                    st2[:, j, :],
                                    lhsT=kT[:, h, kc * 128 : (kc + 1) * 128],
                                    rhs=qT[:, h, q0 : q0 + SC],
                                    start=True,
                                    stop=True,
                                )
                            nc.scalar.activation(pt2, st2, EXP, scale=scale)
                            # NOTE: offloading the pair-sum to gpsimd was
                            # tried and reverted -- gpsimd needs ~1.15us per
                            # [128,512] add vs the 860ns pair cadence, so it
                            # falls behind and the in-order DVE queue ends up
                            # head-blocked on it (+6us).
                            if pr == 1:
                                mid_chain()
                            else:
                                dose()
                            if lst[0]:
                                nc.vector.tensor_add(lall, lall, pt2[:, 0, :])
                            else:
                                nc.vector.tensor_add(
                                    lall, pt2[:, 0, :], pt2[:, 1, :]
                                )
                                lst[0] = True
                            if pr > 0:
                                nc.vector.tensor_add(lall, lall, pt2[:, 1, :])
                            pend_av.append((pr * 2, pt2[:, 0, :], 0))
                            pend_av.append((pr * 2 + 1, pt2[:, 1, :], 0))
                            flush_av(4)

                        # --- diagonal chunks, trimmed + masked ---
                        for di in range(4):
                            kc = nfull + di
                            co = 128 * di
                            st2 = ps_t.tile([128, 2, SC], f32, tag="st")
                            pt2 = p2.tile([128, 2, SC], f16, tag="pt")
                            nc.tensor.matmul(
                                st2[:, 0, co:],
                                lhsT=kT[:, h, kc * 128 : (kc + 1) * 128],
                                rhs=qT[:, h, q0 + co : q0 + SC],
                                start=True,
                                stop=True,
                            )
                            nc.scalar.activation(
                                pt2[:, 0, co:], st2[:, 0, co:], EXP, scale=scale
                            )
                            # causal mask issued BEFORE the dose: otherwise
                            # it queues behind the dosed og eviction on the
                            # DVE and stalls the AV matmul ~760ns per chain
                            # (gpsimd is worse: ~400-600ns op latency)
                            nc.vector.tensor_mul(
                                pt2[:, 0, co : co + 128],
                                pt2[:, 0, co : co + 128],
                                tri01,
                            )
                            if nfull == 0 and di == 1:
                                mid_chain()
                            else:
                                dose()
                            lacc(pt2[:, 0, co:], co)
                            pend_av.append((kc, pt2[:, 0, co:], co))
                            flush_av(4)
                        flush_av(0)
                        if og_gen is not None:
                            for _ in og_gen:
                                pass
                        epi_pend.append((outp, lall, h, q0))
                        if h == HPC - 1:
                            # flush the block's last epilogue here (the
                            # trailing AV flush + og drain above give the
                            # DVE time to finish lall), so the next block's
                            # output-projection doses read normalized uT
                            flush_epi()
                flush_epi()
                n_og = n_oc // 4
                for og in range(n_og):
                    phase3_og_final(qc_order[-1], og, last=(og == n_og - 1))

    nc.compile()
    return nc


def make_in_maps(x, Wq, Wk, Wv, Wo):
    cosT, sinT = _rope_tables_T(S, HD)
    # rotate-half signs folded into the sin table: t2[d] = raw[perm(d)]*sin'[d]
    sinT_mod = sinT.copy()
    sinT_mod[:64] *= np.float16(-1)
    ones = np.ones((HD, HD), dtype=np.float16)
    tri = _tri01()
    n_din, n_sc = DIM // 128, S // SC
    xts = []
    for g in range(DP):
        xT = x[g].T.astype(np.float16)                      # [din, s]
        xts.append(np.ascontiguousarray(
            xT.reshape(n_din, 128, n_sc, SC).transpose(1, 2, 0, 3)
        ))                                                  # [128, si, c, j]
    in_maps = []
    for c in range(N_CORES):
        g, r = divmod(c, TP)
        sl = slice(r * DLOC, (r + 1) * DLOC)

        def tile_w_c(W):
            # [p, c, d] = W.T[c*128+p, d]  (c-major; d = h*HD+dd local dim)
            wT = W[sl, :].T.astype(np.float16)              # [din, dloc]
            return np.ascontiguousarray(
                wT.reshape(n_din, 128, DLOC).transpose(1, 0, 2)
            )

        woT = Wo[:, sl].T.astype(np.float16)                # [dloc, dim]
        wo_t = np.ascontiguousarray(
            woT.reshape(HPC, 128, DIM).transpose(1, 0, 2)
        )
        in_maps.append(
            {
                "x": xts[g],
                "wq": tile_w_c(Wq),
                "wk": tile_w_c(Wk),
                "wv": tile_w_c(Wv),
                "wo": wo_t,
                "cosT": cosT,
                "sinT": sinT_mod,
                "ones": ones,
                "tri": tri,
            }
        )
    return in_maps


def kernel(x, Wq, Wk, Wv, Wo, _trace=False):
    """Full-input / full-output entry point. Shards over 8 cores internally."""
    if "/opt/trn_rl_repo" not in sys.path:
        sys.path.insert(0, "/opt/trn_rl_repo")
    from concourse.bass_utils import run_bass_kernel_spmd

    x = np.asarray(x, dtype=np.float32)
    Wq, Wk, Wv, Wo = (np.asarray(w, dtype=np.float32) for w in (Wq, Wk, Wv, Wo))

    key = (B, S, DIM)
    if key not in _PROGRAM_CACHE:
        _PROGRAM_CACHE[key] = build_program(S, DIM)
    nc = _PROGRAM_CACHE[key]

    in_maps = make_in_maps(x, Wq, Wk, Wv, Wo)
    res = run_bass_kernel_spmd(
        nc, in_maps, core_ids=list(range(N_CORES)), trace=_trace
    )
    kernel.last_results = res
    out = np.empty((B, S, DIM), dtype=np.float32)
    for g in range(DP):
        acc = res.results[g * TP]["out"].astype(np.float32)
        for r in range(1, TP):
            acc = acc + res.results[g * TP + r]["out"].astype(np.float32)
        # [128, qc, oc, j] -> [oc*128, qc*512]
        outT = acc.transpose(2, 0, 1, 3).reshape(DIM, S)
        out[g] = outT.T
    return out


# revision 39
# speedup vs baseline: 1.1964x; 1.0088x over previous
"""Multi-head causal self-attention with RoPE on 8 Trainium2 NeuronCores.

Sharding: DP(2) x TP(4). Cores [4g, 4g+4) own batch g; within a group,
core r owns heads [4r, 4r+4) (rows [r*512,(r+1)*512) of Wq/Wk/Wv and the
matching columns of Wo). The host sums the 4 partial output projections
per batch (replaces the TP all-reduce); partial sums travel as fp16.

Performance notes (measured on TRN2):
  - PE matmul issue rate is N cycles @2.4GHz regardless of operand dtype;
    the kernel is PE-streaming-bound (~280us of columns), so everything
    else is organized to keep the PE FIFO dense.
  - The HAM clock gate runs the PE at 1.2GHz until ~3.4us of sustained
    busy; ~8 dummy matmuls at kernel start warm it up during the initial
    DMA wait (framework preamble ends ~9.2us, first DMA data ~11.4us).
  - dma_start issue costs ~600ns on the Sync queue regardless of size,
    and only 8 HW-DMA semaphores rotate, so transfers are kept >=256KB
    and issued in exact consumption order (wq/x interleaved by c-chunk,
    then wk, wv, xt1, xt2; wv BEFORE xt1 -- the v chains run before the
    si=1 q chains and a late wv costs a 3.9us stall + a HAM re-throttle).
  - s-chunk 0's q/k projections run c-major across 4 concurrent PSUM
    accumulators so the PE starts on the first 256KB of weights/x.
  - Scalar activations pay a ~370-cycle access-latency adder, so exps are
    batched two k-chunks per call; in attention the scalar engine does
    ONLY exps (evictions go to vector/gpsimd) so a score-PSUM tile is
    freed ~780ns after its pair completes and the 3-deep st ring never
    stalls the PE.
  - Softmax denominators: all exp'd chunks of a (head, q-chunk) chain
    are accumulated in fp16 on the DVE, then partition-reduced with a
    single ones-matmul; the reduce/reciprocal/normalize epilogue is
    lagged into the next chain (flushed after its SECOND score pair so
    the DVE accumulation has fully drained), except each block's h=3
    epilogue which flushes at that chain's end so the next block's
    output-projection doses read normalized uT.
  - Attention blocks run qc order [0,3,2,1]: the undosed first block is
    the one with the smallest scalar-exp-over-PE deficit (qc=0,
    diag-only) and needs only s-chunk-0 q/k at the phase boundary.
  - Causality: diagonal-band score chunks are column-trimmed to
    N = 512-128*di and masked multiplicatively (after exp) with a single
    [128,128] triangular 0/1 mask.
"""

import sys

import numpy as np

B, S, DIM = 2, 2048, 2048
NUM_HEADS = 16
HD = 128
N_CORES = 8
DP = 2                       # data-parallel groups (one batch each)
TP = N_CORES // DP           # tensor-parallel ranks per group
HPC = NUM_HEADS // TP        # heads per core (4)
DLOC = HPC * HD              # per-core slice of the model dim (512)
ROPE_BASE = 10000.0
SC = 512                     # s-chunk for projections / attention q-chunk

_PROGRAM_CACHE = {}


def _rope_tables_T(seq_len, head_dim):
    # match reference float32 arithmetic: inv_freq over even indices,
    # emb = cat(freqs, freqs); returned transposed [head_dim, seq_len]
    inv_freq = (
        1.0
        / (np.float32(ROPE_BASE)
           ** (np.arange(0, head_dim, 2, dtype=np.float32) / np.float32(head_dim)))
    ).astype(np.float32)
    t = np.arange(seq_len, dtype=np.float32)
    freqs = np.outer(t, inv_freq).astype(np.float32)      # [S, D/2]
    emb = np.concatenate([freqs, freqs], axis=-1)         # [S, D]
    return (
        np.ascontiguousarray(np.cos(emb).astype(np.float16).T),
        np.ascontiguousarray(np.sin(emb).astype(np.float16).T),
    )


def _tri01():
    # tri01[kk, qq] = 1 if kk <= qq else 0 (multiplicative causal mask for
    # the [128,128] diagonal block of every diagonal k-chunk)
    kk = np.arange(128)[:, None]
    qq = np.arange(128)[None, :]
    return np.ascontiguousarray((kk <= qq).astype(np.float16))


def build_program(s=S, dim=DIM):
    """Per-core SPMD Bass program (identical on every core)."""
    if "/opt/trn_rl_repo" not in sys.path:
        sys.path.insert(0, "/opt/trn_rl_repo")
    import concourse.bacc as bacc
    import concourse.mybir as mybir
    import concourse.tile as tile

    f32 = mybir.dt.float32
    f16 = mybir.dt.float16
    EXP = mybir.ActivationFunctionType.Exp

    n_din = dim // 128          # contraction chunks for projections (16)
    n_sc = s // SC              # s-chunks (4)
    n_oc = dim // 128           # output-projection row chunks (16)
    scale = float(HD) ** -0.5

    nc = bacc.Bacc("TRN2", target_bir_lowering=False, debug=False)

    # all DRAM tensors pre-tiled on the host: partition dim first, then
    # per-partition-contiguous free dims, so DMAs are 128 fat descriptors
    x_d = nc.dram_tensor("x", [128, n_sc, n_din, SC], f16, kind="ExternalInput")
    wq_d = nc.dram_tensor("wq", [128, n_din, DLOC], f16, kind="ExternalInput")
    wk_d = nc.dram_tensor("wk", [128, n_din, DLOC], f16, kind="ExternalInput")
    wv_d = nc.dram_tensor("wv", [128, n_din, DLOC], f16, kind="ExternalInput")
    wo_d = nc.dram_tensor("wo", [128, HPC, dim], f16, kind="ExternalInput")
    cosT_d = nc.dram_tensor("cosT", [HD, s], f16, kind="ExternalInput")
    sinT_d = nc.dram_tensor("sinT", [HD, s], f16, kind="ExternalInput")
    ones_d = nc.dram_tensor("ones", [HD, HD], f16, kind="ExternalInput")
    tri_d = nc.dram_tensor("tri", [HD, HD], f16, kind="ExternalInput")
    out_d = nc.dram_tensor("out", [128, n_sc, n_oc, SC], f16, kind="ExternalOutput")

    with tile.TileContext(nc) as tc:
        with tc.tile_pool(name="persist", bufs=1) as persist:
            qT = persist.tile([128, HPC, s], f16)   # roped q, [d, h, s]
            kT = persist.tile([128, HPC, s], f16)
            vS = persist.tile([128, s // 128, DLOC], f16)  # [k, chunk, d]
            uT = persist.tile([128, HPC, s], f16)   # attention out, [d, h, s]

            # ---------------- phase 1: qkv projections + RoPE ----------------
            with (
                tc.tile_pool(name="p1x", bufs=3) as p1x,
                tc.tile_pool(name="p1w", bufs=1) as p1w,
                tc.tile_pool(name="p1t", bufs=2) as p1t,
                tc.tile_pool(name="ps1", bufs=8, space="PSUM") as ps1,
            ):
                xts = [
                    p1x.tile([128, n_din, SC], f16, tag="xt", name=f"xt{si}")
                    for si in range(3)
                ]
                wq_s = p1w.tile([128, n_din, DLOC], f16)
                wk_s = p1w.tile([128, n_din, DLOC], f16)
                wv_s = p1w.tile([128, n_din, DLOC], f16)
                cosT = persist.tile([HD, s], f16)
                sinT = persist.tile([HD, s], f16)
                ones = persist.tile([HD, HD], f16)
                tri01 = persist.tile([HD, HD], f16)
                woT_s = persist.tile([128, HPC, dim], f16)

                # PE warm-up: the HAM clock gate needs ~3.4us of sustained
                # matmul activity to lift the PE from 1.2 to 2.4GHz.  Burn
                # the dead time between the framework preamble (~9.2us) and
                # the first DMA'd operands (~12.5us) on dummy matmuls so the
                # real stream starts warm.
                warm = p1x.tile([128, SC], f16, tag="warm", bufs=1)
                nc.gpsimd.memset(warm, 0.0)
                for wi in range(8):
                    wacc = ps1.tile([128, SC], f32, tag="acc", name=f"warm{wi}")
                    nc.tensor.matmul(
                        wacc, lhsT=warm[:, :128], rhs=warm, start=True, stop=True
                    )

                # DMA issue order == consumption order.  Each dma_start costs
                # ~600ns of Sync-queue issue, so pieces are >=256KB except the
                # leading ones that gate the very first matmuls.
                def dspan(dst, src, lo, hi):
                    nc.sync.dma_start(out=dst[:, lo:hi, :], in_=src[:, lo:hi, :])

                def dq(a, b):
                    dspan(wq_s, wq_d, a, b)
                    nc.sync.dma_start(
                        out=xts[0][:, a:b, :], in_=x_d[:, 0, a:b, :]
                    )

                dq(0, 2)
                dq(2, 4)
                dq(4, 6)
                dq(6, 8)
                # first s-chunk of the RoPE tables (needed by the first rope
                # at ~25us); the rest comes after wk
                nc.sync.dma_start(out=cosT[:, :SC], in_=cosT_d[:, :SC])
                nc.sync.dma_start(out=sinT[:, :SC], in_=sinT_d[:, :SC])
                dq(8, 10)
                dq(10, 12)
                dq(12, 14)
                dspan(wk_s, wk_d, 0, 2)
                dq(14, 16)
                dspan(wk_s, wk_d, 2, 4)
                dspan(wk_s, wk_d, 4, 6)
                dspan(wk_s, wk_d, 6, 8)
                dspan(wk_s, wk_d, 8, 12)
                dspan(wk_s, wk_d, 12, 16)
                nc.sync.dma_start(out=cosT[:, SC:], in_=cosT_d[:, SC:])
                nc.sync.dma_start(out=sinT[:, SC:], in_=sinT_d[:, SC:])
                dspan(wv_s, wv_d, 0, 8)
                dspan(wv_s, wv_d, 8, 16)
                nh = n_din // 2
                nc.sync.dma_start(out=xts[1][:, :nh, :], in_=x_d[:, 1, :nh, :])
                nc.sync.dma_start(out=xts[1][:, nh:, :], in_=x_d[:, 1, nh:, :])
                nc.sync.dma_start(out=xts[2][:, :nh, :], in_=x_d[:, 2, :nh, :])
                nc.sync.dma_start(out=xts[2][:, nh:, :], in_=x_d[:, 2, nh:, :])
                nc.sync.dma_start(out=ones, in_=ones_d[:])
                nc.sync.dma_start(out=tri01, in_=tri_d[:])
                nc.sync.dma_start(out=woT_s, in_=wo_d[:])

                def finish_rope(raw, store, h, s0):
                    # rotate-half as a partition-permuting SBUF->SBUF DMA
                    # (engines cannot cross partitions; the DMA can), with
                    # the rotation signs folded into the host sin table --
                    # saves a 216ns PE matmul per chain and runs the sin
                    # multiply at 2x fp16 DVE rate. Emitted one chain late
                    # so nothing waits on the scalar-engine raw copy.
                    rawp = p1t.tile([128, SC], f16, tag="rp", bufs=2)
                    nc.sync.dma_start(out=rawp[0:64, :], in_=raw[1:128:2, :])
                    nc.sync.dma_start(out=rawp[64:128, :], in_=raw[0:128:2, :])
                    t1 = p1t.tile([128, SC], f16, tag="t1")
                    nc.vector.tensor_mul(t1, raw, cosT[:, s0 : s0 + SC])
                    t2 = p1t.tile([128, SC], f16, tag="t2")
                    nc.vector.tensor_mul(t2, rawp, sinT[:, s0 : s0 + SC])
                    nc.gpsimd.tensor_add(store[:, h, s0 : s0 + SC], t1, t2)

                pending = None

                # --- si=0: c-major q/k blocks across 4 concurrent PSUM
                # accumulators, so the PE only ever waits on the c-chunk of
                # wq/x currently streaming in (the DMA-paced startup).
                for w_s, store in ((wq_s, qT), (wk_s, kT)):
                    accs = [
                        ps1.tile([128, SC], f32, tag="acc", name=f"p0acc{h}")
                        for h in range(HPC)
                    ]
                    for c in range(n_din):
                        for h in range(HPC):
                            nc.tensor.matmul(
                                accs[h],
                                lhsT=w_s[:, c, h * HD : (h + 1) * HD],
                                rhs=xts[0][:, c, :],
                                start=(c == 0),
                                stop=(c == n_din - 1),
                            )
                    for h in range(HPC):
                        raw = p1t.tile([128, SC], f16, tag="raw", bufs=3)
                        nc.scalar.copy(raw, accs[h])
                        if pending is not None:
                            finish_rope(*pending)
                        pending = (raw, store, h, 0)

                for sub in range(SC // 128):   # si=0 v chains
                    vacc = ps1.tile([128, SC], f32, tag="acc", name="vacc0")
                    for c in range(n_din):
                        nc.tensor.matmul(
                            vacc,
                            lhsT=xts[0][:, c, sub * 128 : (sub + 1) * 128],
                            rhs=wv_s[:, c, :],
                            start=(c == 0),
                            stop=(c == n_din - 1),
                        )
                    if pending is not None:
                        finish_rope(*pending)
                        pending = None
                    vdst = vS[:, sub, :]
                    if sub % 2 == 0:
                        nc.scalar.copy(vdst, vacc)
                    else:
                        nc.vector.tensor_copy(vdst, vacc)

                # --- si=1..3: h-major (weights resident; x tiles streamed)
                for si in range(1, n_sc):
                    s0 = si * SC
                    if si < 3:
                        xt = xts[si]
                    else:
                        xt = p1x.tile([128, n_din, SC], f16, tag="xt", name="xt3")
                        nc.sync.dma_start(out=xt, in_=x_d[:, si, :, :])

                    for w_s, store in ((wq_s, qT), (wk_s, kT)):
                        for h in range(HPC):
                            acc = ps1.tile([128, SC], f32, tag="acc")
                            for c in range(n_din):
                                nc.tensor.matmul(
                                    acc,
                                    lhsT=w_s[:, c, h * HD : (h + 1) * HD],
                                    rhs=xt[:, c, :],
                                    start=(c == 0),
                                    stop=(c == n_din - 1),
                                )
                            raw = p1t.tile([128, SC], f16, tag="raw", bufs=3)
                            nc.scalar.copy(raw, acc)
                            if pending is not None:
                                finish_rope(*pending)
                            pending = (raw, store, h, s0)

                    for sub in range(SC // 128):   # v chains
                        vacc = ps1.tile([128, SC], f32, tag="acc", name="vacc")
                        for c in range(n_din):
                            nc.tensor.matmul(
                                vacc,
                                lhsT=xt[:, c, sub * 128 : (sub + 1) * 128],
                                rhs=wv_s[:, c, :],
                                start=(c == 0),
                                stop=(c == n_din - 1),
                            )
                        if pending is not None:
                            finish_rope(*pending)
                            pending = None
                        vdst = vS[:, si * 4 + sub, :]
                        # si=3: keep the scalar queue clear for the first
                        # attention exps right after the phase boundary
                        if sub % 2 == 0 and si < 3:
                            nc.scalar.copy(vdst, vacc)
                        else:
                            nc.vector.tensor_copy(vdst, vacc)

            # ------------- phase 2+3: attention + output projection -------------
            with (
                tc.tile_pool(name="p2", bufs=8) as p2,
                tc.tile_pool(name="p2l", bufs=2) as p2l,
                tc.tile_pool(name="p2r", bufs=2) as p2r,
                tc.tile_pool(name="p3", bufs=2) as p3,
                tc.tile_pool(name="ps_t", bufs=3, space="PSUM") as ps_t,
                tc.tile_pool(name="ps_o", bufs=2, space="PSUM") as ps_o,
            ):
                def phase3_og_gen(qc, og, n_sc_evict=2):
                    # one 4-row-chunk group of the output projection for
                    # q-chunk qc: 16 PE matmuls + psum evictions + 1 DMA.
                    # Yields after each 4-matmul row-chunk so the group can be
                    # dosed through the next q-chunk's attention chains --
                    # keeping per-pair PE work above the scalar exp rate.
                    # Per-row-chunk psum evictions, alternating engines
                    # (gpsimd cannot read PSUM): half-size ops block the
                    # latency-critical exps/masks in the FIFOs half as long.
                    q0 = qc * SC
                    ot4 = p3.tile([128, 4, SC], f16, tag="ot")
                    for op_ in range(2):
                        pos2 = ps_t.tile([128, 2, SC], f32, tag="st", name="pos")
                        for j in range(2):
                            oc = og * 4 + op_ * 2 + j
                            for h in range(HPC):
                                nc.tensor.matmul(
                                    pos2[:, j, :],
                                    lhsT=woT_s[:, h, oc * 128 : (oc + 1) * 128],
                                    rhs=uT[:, h, q0 : q0 + SC],
                                    start=(h == 0),
                                    stop=(h == HPC - 1),
                                )
                            dst = ot4[:, op_ * 2 + j, :]
                            # row r goes to scalar iff r is among the first
                            # n_sc_evict even rows: the dosing block chooses
                            # the split by its own scalar-exp headroom
                            r = op_ * 2 + j
                            if r % 2 == 0 and r // 2 < n_sc_evict:
                                nc.scalar.copy(dst, pos2[:, j, :])
                            else:
                                nc.vector.tensor_copy(dst, pos2[:, j, :])
                            yield
                    nc.sync.dma_start(
                        out=out_d[:, qc, og * 4 : (og + 1) * 4, :], in_=ot4
                    )

                def phase3_og_final(qc, og, last=False):
                    # final-loop variant (no exps in flight): full-pair
                    # evictions on scalar/vector, DMA split per pair so the
                    # last transfer after the last matmul is only 256KB.
                    # The very last pair is evicted per-oc on BOTH engines in
                    # parallel with per-oc DMAs, halving the kernel tail.
                    q0 = qc * SC
                    ot4 = p3.tile([128, 4, SC], f16, tag="ot")
                    for op_ in range(2):
                        pos2 = ps_t.tile([128, 2, SC], f32, tag="st", name="pos")
                        for j in range(2):
                            oc = og * 4 + op_ * 2 + j
                            for h in range(HPC):
                                nc.tensor.matmul(
                                    pos2[:, j, :],
                                    lhsT=woT_s[:, h, oc * 128 : (oc + 1) * 128],
                                    rhs=uT[:, h, q0 : q0 + SC],
                                    start=(h == 0),
                                    stop=(h == HPC - 1),
                                )
                        dst2 = ot4[:, op_ * 2 : op_ * 2 + 2, :]
                        # vector first, scalar last: the scheduler orders the
                        # second eviction after the first's engine tick, so
                        # put the kernel-tail-critical one on the engine
                        # whose predecessor finished earliest
                        if op_ == 0:
                            nc.vector.tensor_copy(dst2, pos2)
                        else:
                            nc.scalar.copy(dst2, pos2)
                        nc.sync.dma_start(
                            out=out_d[:, qc, og * 4 + op_ * 2 : og * 4 + op_ * 2 + 2, :],
                            in_=dst2,
                        )

                # attention q-chunk order: every chain is paced by
                # max(PE, scalar-exp) work; blocks after the first carry a
                # dosed output-projection group (+3.4us PE) from the
                # previously finished chunk, so the undosed FIRST block
                # should be the one with the smallest scalar-over-PE
                # deficit: qc=0 (diag-only, ~2us exp vs 1.3us PE).  qc=0
                # first also needs only s-chunk-0 q/k, making the
                # phase-1 -> phase-2 handoff dependency-free.  Each chain's
                # epilogue (denominator matmul + reciprocal + normalize) is
                # lagged into the next chain so the PE never waits on the
                # DVE accumulation at head boundaries.
                qc_order = [0] + list(range(n_sc - 1, 0, -1))
                epi_pend = []

                def flush_epi():
                    while epi_pend:
                        outp, lall, h, q0 = epi_pend.pop(0)
                        lrep = ps_t.tile(
                            [128, 2, SC], f32, tag="st", name="lrep"
                        )
                        nc.tensor.matmul(
                            lrep[:, 0, :], lhsT=ones, rhs=lall,
                            start=True, stop=True,
                        )
                        rec = p2r.tile([128, SC], f32, tag="rec")
                        nc.vector.reciprocal_approx_fast(rec, lrep[:, 0, :])
                        nc.vector.tensor_mul(uT[:, h, q0 : q0 + SC], outp, rec)

                def qc0_paired_block():
                    # qc=0 (diag-only, undosed, first): chains have ~1.3us
                    # of PE work vs ~2us of exps, so run them PAIRWISE --
                    # two chains' score matmuls and exps pipeline against
                    # each other instead of serializing the exp latency.
                    q0, nkc = 0, 4
                    for hp in (0, 2):
                        state = []
                        for h in (hp, hp + 1):
                            state.append((
                                h,
                                ps_o.tile(
                                    [128, SC], f32, tag="o", name=f"o0_{h}"
                                ),
                                p2l.tile(
                                    [128, SC], f16, tag="lp", name=f"lp0_{h}"
                                ),
                            ))
                        pend = {0: [], 1: []}
                        for di in range(4):
                            co = 128 * di
                            pts = {}
                            for ci, (h, _, _) in enumerate(state):
                                st2 = ps_t.tile([128, 2, SC], f32, tag="st")
                                pt2 = p2.tile([128, 2, SC], f16, tag="pt")
                                nc.tensor.matmul(
                                    st2[:, 0, co:],
                                    lhsT=kT[:, h, di * 128 : (di + 1) * 128],
                                    rhs=qT[:, h, q0 + co : q0 + SC],
                                    start=True,
                                    stop=True,
                                )
                                nc.scalar.activation(
                                    pt2[:, 0, co:], st2[:, 0, co:],
                                    EXP, scale=scale,
                                )
                                nc.vector.tensor_mul(
                                    pt2[:, 0, co : co + 128],
                                    pt2[:, 0, co : co + 128],
                                    tri01,
                                )
                                pts[ci] = pt2
                            if di == 1:
                                flush_epi()
                            for ci, (h, _, lall) in enumerate(state):
                                if di == 0:
                                    nc.vector.tensor_copy(
                                        lall, pts[ci][:, 0, :]
                                    )
                                else:
                                    nc.vector.tensor_add(
                                        lall[:, co:], lall[:, co:],
                                        pts[ci][:, 0, co:],
                                    )
                                pend[ci].append((di, pts[ci][:, 0, co:], co))
                        for ci, (h, outp, lall) in enumerate(state):
                            for kc, ap, co in pend[ci]:
                                nc.tensor.matmul(
                                    outp[:, co:],
                                    lhsT=vS[:, kc, h * HD : (h + 1) * HD],
                                    rhs=ap,
                                    start=(kc == 0),
                                    stop=(kc == nkc - 1),
                                )
                            epi_pend.append((outp, lall, h, q0))
                        if hp == 2:
                            # block-end flush so the next block's og doses
                            # read normalized uT
                            flush_epi()

                for oi, qc in enumerate(qc_order):
                    q0 = qc * SC
                    nfull = 4 * qc          # full (sub-diagonal) k-chunks
                    nkc = nfull + 4
                    prev_qc = qc_order[oi - 1] if oi > 0 else None
                    if oi == 0:
                        qc0_paired_block()
                        continue
                    for h in range(HPC):
                        outp = ps_o.tile([128, SC], f32, tag="o")
                        lall = p2l.tile([128, SC], f16, tag="lp")
                        lst = [False]       # lall initialized?
                        pend_av = []        # (kc, pt AP, co) awaiting AV
                        og_gen = (
                            # the qc=3 block's chains have the least scalar
                            # headroom (8.6us of exps vs 9.8us PE): evict
                            # only 1 of 4 rows on scalar there
                            phase3_og_gen(prev_qc, h, 1 if qc == 3 else 2)
                            if prev_qc is not None
                            else None
                        )

                        def dose():
                            # a slice of the previous q-chunk's output
                            # projection as PE filler between score pairs
                            if og_gen is not None:
                                next(og_gen, None)

                        def mid_chain():
                            # previous chain's epilogue; deferred to the
                            # SECOND score pair so the previous chain's DVE
                            # denominator accumulation has fully drained.
                            # (Safe for the og dose at pr==0: each block's
                            # h=3 epilogue is flushed at that chain's end,
                            # so all uT heads of the previous q-chunk are
                            # normalized before the next block's doses.)
                            flush_epi()
                            dose()

                        def lacc(ap, co):
                            # fp16 DVE accumulation of the softmax denominator
                            if not lst[0]:
                                nc.vector.tensor_copy(lall, ap)
                                lst[0] = True
                            else:
                                nc.vector.tensor_add(
                                    lall[:, co:], lall[:, co:], ap
                                )

                        def flush_av(upto):
                            # AV matmuls lag the score/exp stream to keep exp
                            # latency off the PE critical path
                            while len(pend_av) > upto:
                                kc, pt_ap, co = pend_av.pop(0)
                                nc.tensor.matmul(
                                    outp[:, co:],
                                    lhsT=vS[:, kc, h * HD : (h + 1) * HD],
                                    rhs=pt_ap,
                                    start=(kc == 0),
                                    stop=(kc == nkc - 1),
                                )

                        # --- full chunks, exp'd in pairs ---
                        for pr in range(nfull // 2):
                            st2 = ps_t.tile([128, 2, SC], f32, tag="st")
                            pt2 = p2.tile([128, 2, SC], f16, tag="pt")
                            for j in range(2):
                                kc = pr * 2 + j
                                nc.tensor.matmul(
                                    st2[:, j, :],
                                    lhsT=kT[:, h, kc * 128 : (kc + 1) * 128],
                                    rhs=qT[:, h, q0 : q0 + SC],
                                    start=True,
                                    stop=True,
                                )
                            nc.scalar.activation(pt2, st2, EXP, scale=scale)
                            # NOTE: offloading the pair-sum to gpsimd was
                            # tried and reverted -- gpsimd needs ~1.15us per
                            # [128,512] add vs the 860ns pair cadence, so it
                            # falls behind and the in-order DVE queue ends up
                            # head-blocked on it (+6us).
                            if pr == 1:
                                mid_chain()
                            else:
                                dose()
                            if lst[0]:
                                nc.vector.tensor_add(lall, lall, pt2[:, 0, :])
                            else:
                                nc.vector.tensor_add(
                                    lall, pt2[:, 0, :], pt2[:, 1, :]
                                )
                                lst[0] = True
                            if pr > 0:
                                nc.vector.tensor_add(lall, lall, pt2[:, 1, :])
                            pend_av.append((pr * 2, pt2[:, 0, :], 0))
                            pend_av.append((pr * 2 + 1, pt2[:, 1, :], 0))
                            flush_av(4)

                        # --- diagonal chunks, trimmed + masked ---
                        for di in range(4):
                            kc = nfull + di
                            co = 128 * di
                            st2 = ps_t.tile([128, 2, SC], f32, tag="st")
                            pt2 = p2.tile([128, 2, SC], f16, tag="pt")
                            nc.tensor.matmul(
                                st2[:, 0, co:],
                                lhsT=kT[:, h, kc * 128 : (kc + 1) * 128],
                                rhs=qT[:, h, q0 + co : q0 + SC],
                                start=True,
                                stop=True,
                            )
                            nc.scalar.activation(
                                pt2[:, 0, co:], st2[:, 0, co:], EXP, scale=scale
                            )
                            # causal mask issued BEFORE the dose: otherwise
                            # it queues behind the dosed og eviction on the
                            # DVE and stalls the AV matmul ~760ns per chain
                            # (gpsimd is worse: ~400-600ns op latency)
                            nc.vector.tensor_mul(
                                pt2[:, 0, co : co + 128],
                                pt2[:, 0, co : co + 128],
                                tri01,
                            )
                            if nfull == 0 and di == 1:
                                mid_chain()
                            else:
                                dose()
                            lacc(pt2[:, 0, co:], co)
                            pend_av.append((kc, pt2[:, 0, co:], co))
                            flush_av(5)
                        flush_av(0)
                        if og_gen is not None:
                            for _ in og_gen:
                                pass
                        epi_pend.append((outp, lall, h, q0))
                        if h == HPC - 1:
                            # flush the block's last epilogue here (the
                            # trailing AV flush + og drain above give the
                            # DVE time to finish lall), so the next block's
                            # output-projection doses read normalized uT
                            flush_epi()
                flush_epi()
                n_og = n_oc // 4
                for og in range(n_og):
                    phase3_og_final(qc_order[-1], og, last=(og == n_og - 1))

    nc.compile()
    return nc


def make_in_maps(x, Wq, Wk, Wv, Wo):
    cosT, sinT = _rope_tables_T(S, HD)
    # rotate-half signs folded into the sin table: t2[d] = raw[perm(d)]*sin'[d]
    sinT_mod = sinT.copy()
    sinT_mod[:64] *= np.float16(-1)
    ones = np.ones((HD, HD), dtype=np.float16)
    tri = _tri01()
    n_din, n_sc = DIM // 128, S // SC
    xts = []
    for g in range(DP):
        xT = x[g].T.astype(np.float16)                      # [din, s]
        xts.append(np.ascontiguousarray(
            xT.reshape(n_din, 128, n_sc, SC).transpose(1, 2, 0, 3)
        ))                                                  # [128, si, c, j]
    in_maps = []
    for c in range(N_CORES):
        g, r = divmod(c, TP)
        sl = slice(r * DLOC, (r + 1) * DLOC)

        def tile_w_c(W):
            # [p, c, d] = W.T[c*128+p, d]  (c-major; d = h*HD+dd local dim)
            wT = W[sl, :].T.astype(np.float16)              # [din, dloc]
            return np.ascontiguousarray(
                wT.reshape(n_din, 128, DLOC).transpose(1, 0, 2)
            )

        woT = Wo[:, sl].T.astype(np.float16)                # [dloc, dim]
        wo_t = np.ascontiguousarray(
            woT.reshape(HPC, 128, DIM).transpose(1, 0, 2)
        )
        in_maps.append(
            {
                "x": xts[g],
                "wq": tile_w_c(Wq),
                "wk": tile_w_c(Wk),
                "wv": tile_w_c(Wv),
                "wo": wo_t,
                "cosT": cosT,
                "sinT": sinT_mod,
                "ones": ones,
                "tri": tri,
            }
        )
    return in_maps


def kernel(x, Wq, Wk, Wv, Wo, _trace=False):
    """Full-input / full-output entry point. Shards over 8 cores internally."""
    if "/opt/trn_rl_repo" not in sys.path:
        sys.path.insert(0, "/opt/trn_rl_repo")
    from concourse.bass_utils import run_bass_kernel_spmd

    x = np.asarray(x, dtype=np.float32)
    Wq, Wk, Wv, Wo = (np.asarray(w, dtype=np.float32) for w in (Wq, Wk, Wv, Wo))

    key = (B, S, DIM)
    if key not in _PROGRAM_CACHE:
        _PROGRAM_CACHE[key] = build_program(S, DIM)
    nc = _PROGRAM_CACHE[key]

    in_maps = make_in_maps(x, Wq, Wk, Wv, Wo)
    res = run_bass_kernel_spmd(
        nc, in_maps, core_ids=list(range(N_CORES)), trace=_trace
    )
    kernel.last_results = res
    out = np.empty((B, S, DIM), dtype=np.float32)
    for g in range(DP):
        acc = res.results[g * TP]["out"].astype(np.float32)
        for r in range(1, TP):
            acc = acc + res.results[g * TP + r]["out"].astype(np.float32)
        # [128, qc, oc, j] -> [oc*128, qc*512]
        outT = acc.transpose(2, 0, 1, 3).reshape(DIM, S)
        out[g] = outT.T
    return out
